# revision 30
# baseline (speedup 1.0000x reference)
"""CenterLoss kernel for Trainium2 (8 NeuronCores, data-parallel over batch).

loss = mean_i( clip( ||x_i - centers[labels[i]]||^2, 1e-12, 1e12 ) )

v7 hybrid: the per-sample center-row fetch is split across two independent
engine pipelines so their serial chains run CONCURRENTLY:

  - chunks 0-3 (512 samples): SWDGE indirect gather. Each 128-row call
    costs ~1.4us of GPSIMD ucode (994ns fixed, ring holds only 128
    descriptors), so 4 calls = ~5.6us instead of 8 = ~11.3us.
  - chunks 4-7: one-hot matmul on the otherwise-idle PE. The host sorts
    samples by label (a pure input permutation; the mean is order-
    invariant and per-core ordering is the sharding strategy), so each
    128-sample chunk's labels span < 256 consecutive classes (max 145
    observed for the input spec). The host ships contiguous 256-row
    center windows (slicing only, no value computation); the device
    broadcasts rr = label - base across partitions with a K=1 ones-
    matmul, builds one-hot halves with DVE is_equal against a per-
    partition iota, and selects rows exactly via P^T.T @ W into fp32
    PSUM. Window DMAs ride the Scalar engine's HWDGE queue so they
    don't contend with idx/x on the Sync queue.

  The idx load is issued BEFORE TileContext entry (manual semaphore,
  wait attached to the first gather after tile scheduling) to skip the
  tile entry drains: gather descriptor-gen starts ~1us earlier.

Numerics: x/centers bf16, accumulation fp32 (ACT accumulator / DVE
accumulator): ~1e-3 relative error vs the fp32 reference (gate: 2e-2).
A window-overflow check raises for inputs whose sorted chunks span
>= 256 classes (never observed: worst random-trial span 172).
"""

import sys

import numpy as np

if "/opt/trn_rl_repo" not in sys.path:
    sys.path.insert(0, "/opt/trn_rl_repo")

import ml_dtypes

_B, _D, _C = 8192, 512, 8000
_N_CORES = 8
_B_LOC = _B // _N_CORES  # 1024 rows per core
_P = 128
_M = _B_LOC // _P  # 8 chunks of 128 rows
_MS = 4  # chunks gathered via SWDGE (0.._MS)
_MW = _M - _MS  # chunks selected via PE windows (_MS.._M)
_W = 256  # center window rows per chunk
_CLAMP_MIN, _CLAMP_MAX = 1e-12, 1e12

_cache: dict = {}


def _build():
    import concourse.bass as bass
    import concourse.tile as tile
    from concourse import bacc, mybir

    nc = bacc.Bacc(
        "TRN2",
        debug=False,
        enable_asserts=False,
        target_bir_lowering=False,
        num_devices=_N_CORES,
    )
    # x chunk-major: x_d[p, t*512:(t+1)*512] = x_sorted[t*128+p], bf16
    x_d = nc.dram_tensor("x", [_P, _M * _D], mybir.dt.bfloat16, kind="ExternalInput")
    # SWDGE offsets: idx[p, m] = label_sorted[m*128+p], int32, chunks 0-3
    lab_d = nc.dram_tensor("labels_packed", [_P, _MS], mybir.dt.int32, kind="ExternalInput")
    cen_d = nc.dram_tensor("centers", [_C, _D], mybir.dt.bfloat16, kind="ExternalInput")
    # window stacks for chunks 4-7: w_d[p, ((t-4)*2+u)*512+d] =
    # centers[base_t + u*128 + p][d], bf16
    w_d = nc.dram_tensor("wins", [_P, _MW * 2 * _D], mybir.dt.bfloat16, kind="ExternalInput")
    # rr[0, (t-4)*128+j] = label_sorted[t*128+j] - base_t (< 256, exact bf16)
    rr_d = nc.dram_tensor("rr", [1, _MW * _P], mybir.dt.bfloat16, kind="ExternalInput")
    out_d = nc.dram_tensor("out", [_P, _M], mybir.dt.float32, kind="ExternalOutput")

    # idx load before TileContext entry: skips the tile entry drains so
    # gather descriptor-gen starts as early as possible. Hand-synced via
    # idx_sem, attached to the first gather after tile scheduling.
    idx_sb = nc.alloc_sbuf_tensor("idx_early", [_P, _MS], mybir.dt.int32)
    idx_sem = nc.alloc_semaphore("idx_sem")
    nc.sync.dma_start(out=idx_sb[:], in_=lab_d.ap()).then_inc(idx_sem, 16)

    with tile.TileContext(nc) as tc:
        with (
            tc.tile_pool(name="big", bufs=1) as big,
            tc.tile_pool(name="work", bufs=4) as work,
            tc.tile_pool(name="misc", bufs=1) as misc,
            tc.tile_pool(name="psum_rr", bufs=1, space="PSUM") as psum_rr,
            tc.tile_pool(name="psum_g", bufs=4, space="PSUM") as psum_g,
        ):
            gather_h = []

            # window-path constants (cheap, off the critical path)
            ones = misc.tile([1, _P], mybir.dt.bfloat16)
            nc.gpsimd.memset(ones[:], 1.0)
            iota_col = misc.tile([_P, 1], mybir.dt.float32)
            nc.gpsimd.iota(
                iota_col[:], pattern=[[0, 1]], base=0, channel_multiplier=1,
                allow_small_or_imprecise_dtypes=True,
            )

            # Sync queue: rr (1 descriptor), then x for the SWDGE chunks;
            # Scalar HWDGE queue: wins, then x for the window chunks. Each
            # queue only carries what its pipeline needs early, so the
            # gather ring transfers aren't starved by bulk traffic.
            rrsb = misc.tile([1, _MW * _P], mybir.dt.bfloat16)
            nc.sync.dma_start(out=rrsb[:], in_=rr_d.ap())
            xsb = big.tile([_P, _M * _D], mybir.dt.bfloat16)
            halfx = _MS * _D
            nc.sync.dma_start(out=xsb[:, :halfx], in_=x_d.ap()[:, :halfx])
            wsb = big.tile([_P, _MW * 2 * _D], mybir.dt.bfloat16)
            nc.scalar.dma_start(out=wsb[:], in_=w_d.ap())
            nc.scalar.dma_start(out=xsb[:, halfx:], in_=x_d.ap()[:, halfx:])

            # broadcast rr across partitions: rrb[p, i] = rr[i], fp32 exact
            rrb = psum_rr.tile([_P, _MW * _P], mybir.dt.float32)
            nc.tensor.matmul(rrb[:], ones[:], rrsb[:], start=True, stop=True)

            # one-hot halves for all window chunks: pt_u[j, i] = (rr[i]==u*128+j)
            pt0 = misc.tile([_P, _MW * _P], mybir.dt.bfloat16)
            nc.vector.tensor_scalar(
                out=pt0[:], in0=rrb[:], scalar1=iota_col[:], scalar2=None,
                op0=mybir.AluOpType.is_equal,
            )
            pt1 = misc.tile([_P, _MW * _P], mybir.dt.bfloat16)
            nc.vector.tensor_scalar(
                out=pt1[:], in0=rrb[:], scalar1=float(_P), scalar2=iota_col[:],
                op0=mybir.AluOpType.subtract, op1=mybir.AluOpType.is_equal,
            )

            dist = misc.tile([_P, _M], mybir.dt.float32)

            g = big.tile([_P, _MS * _D], mybir.dt.bfloat16)
            g3 = g[:].rearrange("p (m d) -> p m d", d=_D)

            def rowsum(t, diff, on_dve):
                if not on_dve:
                    sq = work.tile([_P, _D], mybir.dt.bfloat16, tag="sq")
                    nc.scalar.activation(
                        out=sq[:], in_=diff[:],
                        func=mybir.ActivationFunctionType.Square,
                        accum_out=dist[:, t : t + 1],
                    )
                else:
                    sq = work.tile([_P, _D], mybir.dt.bfloat16, tag="sqv")
                    nc.vector.scalar_tensor_tensor(
                        out=sq[:], in0=diff[:], scalar=0.0, in1=diff[:],
                        op0=mybir.AluOpType.bypass, op1=mybir.AluOpType.mult,
                        accum_out=dist[:, t : t + 1],
                    )

            _DVE_SQ = {4, 6}  # chunks whose square+rowsum runs on DVE

            # interleave: SWDGE chunk m and window chunk 4+m alternate in
            # program order so every engine pipelines across both paths.
            for m in range(_MS):
                h = nc.gpsimd.indirect_dma_start(
                    out=g3[:, m, :],
                    out_offset=None,
                    in_=cen_d.ap(),
                    in_offset=bass.IndirectOffsetOnAxis(
                        ap=idx_sb[:, m : m + 1], axis=0
                    ),
                )
                gather_h.append(h)

                # window chunk t = 4 + m
                t = _MS + m
                k = m  # window index
                gt = psum_g.tile([_P, _D], mybir.dt.float32, tag="g")
                nc.tensor.matmul(
                    gt[:], pt0[:, k * _P : (k + 1) * _P],
                    wsb[:, (2 * k) * _D : (2 * k + 1) * _D],
                    start=True, stop=False,
                )
                nc.tensor.matmul(
                    gt[:], pt1[:, k * _P : (k + 1) * _P],
                    wsb[:, (2 * k + 1) * _D : (2 * k + 2) * _D],
                    start=False, stop=True,
                )

                # SWDGE chunk m compute
                diff = work.tile([_P, _D], mybir.dt.bfloat16, tag="diff")
                nc.vector.tensor_tensor(
                    out=diff[:],
                    in0=xsb[:, m * _D : (m + 1) * _D],
                    in1=g[:, m * _D : (m + 1) * _D],
                    op=mybir.AluOpType.subtract,
                )
                rowsum(m, diff, m in _DVE_SQ)

                # window chunk t compute
                diffw = work.tile([_P, _D], mybir.dt.bfloat16, tag="diffw")
                nc.vector.tensor_tensor(
                    out=diffw[:],
                    in0=xsb[:, t * _D : (t + 1) * _D],
                    in1=gt[:],
                    op=mybir.AluOpType.subtract,
                )
                rowsum(t, diffw, t in _DVE_SQ)

            # clip both bounds in one DVE op: out = min(max(dist, lo), hi).
            nc.vector.tensor_scalar(
                out=dist[:, : _M - 1],
                in0=dist[:, : _M - 1],
                scalar1=_CLAMP_MIN,
                scalar2=_CLAMP_MAX,
                op0=mybir.AluOpType.max,
                op1=mybir.AluOpType.min,
            )
            nc.vector.tensor_scalar(
                out=dist[:, _M - 1 :],
                in0=dist[:, _M - 1 :],
                scalar1=_CLAMP_MIN,
                scalar2=_CLAMP_MAX,
                op0=mybir.AluOpType.max,
                op1=mybir.AluOpType.min,
            )

            nc.sync.dma_start(out=out_d.ap()[:, :], in_=dist[:])
    # Attach the idx-DMA wait after tile scheduling (the scheduler's
    # block-local sim cannot see the pre-tile DMA's increment).
    gather_h[0].wait_op(idx_sem, 16, "sem-ge")
    nc.compile()
    return nc


def _prep_core(x_bf16_sorted, labels_sorted, centers_bf16, c):
    """Build one core's in_map from the globally sorted arrays.

    Returns None if any window chunk's label span exceeds 256 rows.
    """
    sl = slice(c * _B_LOC, (c + 1) * _B_LOC)
    xs = x_bf16_sorted[sl]  # [1024, 512]
    ls = labels_sorted[sl]  # [1024] (sorted)

    lab = ls.reshape(_M, _P)  # [t, p]: sample (t, p) = index t*128+p

    wins = np.empty((_MW, 2, _P, _D), dtype=centers_bf16.dtype)
    rr = np.empty(_MW * _P, dtype=np.float32)
    for k in range(_MW):
        chunk = lab[_MS + k]
        base = min(int(chunk[0]), _C - _W)
        if int(chunk[-1]) - base >= _W:
            return None
        wins[k] = centers_bf16[base : base + _W].reshape(2, _P, _D)
        rr[k * _P : (k + 1) * _P] = chunk - base

    return {
        "x": np.ascontiguousarray(
            xs.reshape(_M, _P, _D).transpose(1, 0, 2).reshape(_P, _M * _D)
        ),
        "centers": centers_bf16,
        "labels_packed": np.ascontiguousarray(lab[:_MS].T.astype(np.int32)),
        "wins": np.ascontiguousarray(
            wins.transpose(2, 0, 1, 3).reshape(_P, -1)
        ),
        "rr": np.ascontiguousarray(
            rr.astype(ml_dtypes.bfloat16).reshape(1, -1)
        ),
    }


def _run(x, labels, centers, trace=False, **hw_kwargs):
    from concourse import bass_utils

    if "nc" not in _cache:
        _cache["nc"] = _build()
    nc = _cache["nc"]

    x = np.asarray(x, dtype=np.float32).astype(ml_dtypes.bfloat16)
    labels = np.asarray(labels).astype(np.int64)
    centers = np.ascontiguousarray(
        np.asarray(centers, dtype=np.float32).astype(ml_dtypes.bfloat16)
    )
    assert x.shape == (_B, _D) and labels.shape == (_B,) and centers.shape == (_C, _D)
    assert labels.min() >= 0 and labels.max() < _C

    order = np.argsort(labels, kind="stable")
    x_sorted = x[order]
    labels_sorted = labels[order]

    in_maps = []
    for c in range(_N_CORES):
        m = _prep_core(x_sorted, labels_sorted, centers, c)
        if m is None:
            raise RuntimeError("window overflow — SWDGE fallback required")
        in_maps.append(m)

    r = bass_utils.run_bass_kernel_spmd(
        nc, in_maps, core_ids=list(range(_N_CORES)), trace=trace, **hw_kwargs
    )
    total = sum(res["out"].astype(np.float64).sum() for res in r.results)
    return np.array(total / _B, dtype=np.float32), r


def kernel(x, labels, centers):
    out, _ = _run(x, labels, centers, trace=False)
    return out


# revision 31
# speedup vs baseline: 1.0405x; 1.0405x over previous
"""CenterLoss kernel for Trainium2 (8 NeuronCores, data-parallel over batch).

loss = mean_i( clip( ||x_i - centers[labels[i]]||^2, 1e-12, 1e12 ) )

Gather the labeled center row per sample with indirect DMA and compute the
squared distance directly: O(B*D) work instead of O(B*C*D).

Sharding: x/labels split into 8 batch shards of 1024 rows; centers replicated.
Host sums the 8 partial outputs and divides by global B.

Perf notes (v5):
  - The SWDGE ring holds only 128 in-flight descriptors per queue, so the
    gather is 8 indirect calls x 128 rows; merged calls overflow the ring
    and serialize at ~320ns/descriptor. Calls alternate between two SWDGE
    queues (num_swdge_queues=2, queue patched on the emitted InstDMACopy)
    so call k+1's descriptor-gen never waits for call k's ring to drain,
    and the two rings' transfers overlap.
  - x and centers ship as bf16 (host-converted): halves every DMA byte and
    doubles DVE throughput. Per-sample accumulation stays fp32 (ACT
    accumulator / fp32 reduce outputs): ~1e-3 relative error vs the fp32
    reference, far inside the 2e-2 gate.
  - The idx load is split: a 1-column DMA unblocks gather 0's descriptor
    generation ~1us earlier; columns 1-7 follow in a second DMA that lands
    before gather 1 needs them.

Per-core layout (B_loc=1024, P=128 partitions, M=8 row-chunks):
  sample s lives at (partition p, chunk m) with s = p*8 + m; the x DMA
  reads 8KB contiguous bf16 per partition. idx[p, m] = labels[p*8+m] int32;
  gather call m uses offset AP idx[:, m] (per-partition column — a
  single-partition-row offset AP crashes the HW SWDGE).
"""

import sys

import numpy as np

if "/opt/trn_rl_repo" not in sys.path:
    sys.path.insert(0, "/opt/trn_rl_repo")

import ml_dtypes

_B, _D, _C = 8192, 512, 8000
_N_CORES = 8
_B_LOC = _B // _N_CORES  # 1024 rows per core
_P = 128
_M = _B_LOC // _P  # 8 chunks of 128 rows
_N_QUEUES = 2
_CLAMP_MIN, _CLAMP_MAX = 1e-12, 1e12

_cache: dict = {}


def _build():
    import concourse.bass as bass
    import concourse.tile as tile
    from concourse import bacc, mybir

    nc = bacc.Bacc(
        "TRN2",
        debug=False,
        enable_asserts=False,
        target_bir_lowering=False,
        num_devices=_N_CORES,
        num_swdge_queues=_N_QUEUES,
    )
    x_d = nc.dram_tensor("x", [_B_LOC, _D], mybir.dt.bfloat16, kind="ExternalInput")
    # labels arrive host-packed as idx[p, m] = labels[p*8 + m] (see kernel()).
    lab_d = nc.dram_tensor("labels_packed", [_P, _M], mybir.dt.int32, kind="ExternalInput")
    cen_d = nc.dram_tensor("centers", [_C, _D], mybir.dt.bfloat16, kind="ExternalInput")
    out_d = nc.dram_tensor("out", [_P, _M], mybir.dt.float32, kind="ExternalOutput")

    # Issue the idx load BEFORE TileContext entry: it skips the tile
    # framework's entry drains/ordering, shaving ~1us off the critical
    # path (all gather descriptor-gen waits on this DMA). Hand-synced via
    # idx_sem -> gpsimd.wait_ge before the first indirect call.
    idx_sb = nc.alloc_sbuf_tensor("idx_early", [_P, _M], mybir.dt.int32)
    idx_sem = nc.alloc_semaphore("idx_sem")
    nc.sync.dma_start(out=idx_sb[:], in_=lab_d.ap()).then_inc(idx_sem, 16)

    with tile.TileContext(nc) as tc:
        with (
            tc.tile_pool(name="big", bufs=1) as big,
            tc.tile_pool(name="work", bufs=4) as work,
            tc.tile_pool(name="misc", bufs=1) as misc,
        ):
            idx = idx_sb
            gather_h = []

            xsb = big.tile([_P, _M * _D], mybir.dt.bfloat16)
            nc.sync.dma_start(
                out=xsb[:], in_=x_d.ap().rearrange("(p m) d -> p (m d)", p=_P)
            )

            dist = misc.tile([_P, _M], mybir.dt.float32)

            g = big.tile([_P, _M * _D], mybir.dt.bfloat16)
            g3 = g[:].rearrange("p (m d) -> p m d", d=_D)
            _DVE_SQ = {4, 6}  # chunks whose square+rowsum runs on DVE
            for m in range(_M):
                h = nc.gpsimd.indirect_dma_start(
                    out=g3[:, m, :],
                    out_offset=None,
                    in_=cen_d.ap(),
                    in_offset=bass.IndirectOffsetOnAxis(
                        ap=idx[:, m : m + 1], axis=0
                    ),
                )
                # Alternate SWDGE queues so ring await_space never stalls
                # the next call's descriptor generation.
                if m % _N_QUEUES:
                    h.ins.queue = "qPoolDynamic1"
                gather_h.append(h)
                diff = work.tile([_P, _D], mybir.dt.bfloat16, tag="diff")
                nc.vector.tensor_tensor(
                    out=diff[:],
                    in0=xsb[:, m * _D : (m + 1) * _D],
                    in1=g[:, m * _D : (m + 1) * _D],
                    op=mybir.AluOpType.subtract,
                )
                if m not in _DVE_SQ:
                    # fused square + fp32 row-sum on the scalar engine
                    sq = work.tile([_P, _D], mybir.dt.bfloat16, tag="sq")
                    nc.scalar.activation(
                        out=sq[:],
                        in_=diff[:],
                        func=mybir.ActivationFunctionType.Square,
                        accum_out=dist[:, m : m + 1],
                    )
                else:
                    # balance engines: one fused DVE op — square via
                    # (diff bypass) * diff with free-axis accumulation
                    sq = work.tile([_P, _D], mybir.dt.bfloat16, tag="sqv")
                    nc.vector.scalar_tensor_tensor(
                        out=sq[:],
                        in0=diff[:],
                        scalar=0.0,
                        in1=diff[:],
                        op0=mybir.AluOpType.bypass,
                        op1=mybir.AluOpType.mult,
                        accum_out=dist[:, m : m + 1],
                    )

            # clip both bounds in one DVE op: out = min(max(dist, lo), hi).
            # Columns 0-6 clip as soon as chunk 6 lands; only column 7's tiny
            # clip trails the final accum, so the out-DMA fires sooner.
            nc.vector.tensor_scalar(
                out=dist[:, : _M - 1],
                in0=dist[:, : _M - 1],
                scalar1=_CLAMP_MIN,
                scalar2=_CLAMP_MAX,
                op0=mybir.AluOpType.max,
                op1=mybir.AluOpType.min,
            )
            nc.vector.tensor_scalar(
                out=dist[:, _M - 1 :],
                in0=dist[:, _M - 1 :],
                scalar1=_CLAMP_MIN,
                scalar2=_CLAMP_MAX,
                op0=mybir.AluOpType.max,
                op1=mybir.AluOpType.min,
            )

            nc.sync.dma_start(out=out_d.ap()[:, :], in_=dist[:])
    # Attach the idx-DMA wait to the first gather AFTER tile scheduling
    # (the scheduler's block-local sim cannot see the pre-tile DMA's
    # increment and would deadlock on an in-block wait).
    gather_h[0].wait_op(idx_sem, 16, "sem-ge")
    nc.compile()
    return nc


def _pack_labels(labels_shard: np.ndarray) -> np.ndarray:
    """idx[p, m] = labels[p*8 + m], int32, shape [128, 8]."""
    return np.ascontiguousarray(labels_shard.reshape(_P, _M).astype(np.int32))


def _run(x, labels, centers, trace=False, **hw_kwargs):
    from concourse import bass_utils

    if "nc" not in _cache:
        _cache["nc"] = _build()
    nc = _cache["nc"]

    x = np.asarray(x, dtype=np.float32).astype(ml_dtypes.bfloat16)
    labels = np.ascontiguousarray(np.asarray(labels).astype(np.int64))
    centers = np.ascontiguousarray(
        np.asarray(centers, dtype=np.float32).astype(ml_dtypes.bfloat16)
    )
    assert x.shape == (_B, _D) and labels.shape == (_B,) and centers.shape == (_C, _D)
    assert labels.min() >= 0 and labels.max() < _C

    in_maps = []
    for c in range(_N_CORES):
        sl = slice(c * _B_LOC, (c + 1) * _B_LOC)
        in_maps.append(
            {
                "x": np.ascontiguousarray(x[sl]),
                "labels_packed": _pack_labels(labels[sl]),
                "centers": centers,
            }
        )

    r = bass_utils.run_bass_kernel_spmd(
        nc, in_maps, core_ids=list(range(_N_CORES)), trace=trace, **hw_kwargs
    )
    total = sum(res["out"].astype(np.float64).sum() for res in r.results)
    return np.array(total / _B, dtype=np.float32), r


def kernel(x, labels, centers):
    out, _ = _run(x, labels, centers, trace=False)
    return out
